# revision 24
# baseline (speedup 1.0000x reference)
"""NlmCNN (weight-predicting CNN + per-pixel 13x13 weighted sum) on 8 trn2 cores.

Sharding: data-parallel over batch (8 images -> 8 cores), weights replicated.

Per-core layout trick: output y is the conv stack's result center-cropped by
6 pixels, and the receptive field of the three 3x3 convs only reaches 3 px
out, so SAME-padding never materializes: every layer is computed VALID-style
on an unpadded 256-stride flat layout. Column-wrap junk from flat shifted
reads stays confined to the outer <=3 columns of each layer, which are
discarded by the crop.

Pipeline per strip of S output rows:
  conv1: per-2-chunk im2col [9, 1024] via one 3-dim DMA -> K=9 matmul.
         DMAs for strip i+1 are issued at the START of strip i (so the
         transfers never race the strip-boundary traffic); the matmuls
         are interleaved 1-2 at a time into strip i's conv3 phase so the
         PE never sits in an ACT-throughput-bound conv1 burst (which
         would trip the HAM throttle down to half clock).
  conv2/conv3: taps (du,0)+(du,1) fused into K=128 matmuls against an SBUF
         tile whose upper 64 partitions hold h shifted by +1 (built by two
         SBUF->SBUF SWDGE DMAs per strip); taps (du,2) are K=64 singles.
         conv3's 169 output channels split 128+41 (full-M matmuls first).
  einsum: patch matrix xs[c, s] = x[pos + shift(c)] via per-chunk strided
         DMAs (6-deep pool, so transfers spread across the strip);
         DVE scalar_tensor_tensor computes t = (conv3_psum + b3) * xs from
         PSUM; the 41 hi channels are DVE-folded into the lo tile, then a
         single K=128 "staircase ones" matmul per chunk (deferred so the
         PE never waits on the DVE) reduces over partitions, accumulating
         2-row chunk j into row j of one persistent PSUM tile; one copy +
         one DMA store the whole image.

All matmul inputs are float32r (fp32 streamed at bf16 rate, ~1e-4 relerr).
"""

import numpy as np

import concourse.bacc as bacc
import concourse.bass as bass
import concourse.mybir as mybir
import concourse.tile as tile
from concourse.bass_utils import run_bass_kernel_spmd

F32 = mybir.dt.float32
F32R = mybir.dt.float32r
AF = mybir.ActivationFunctionType
ALU = mybir.AluOpType

H = 256
W = 256
K = 13
HO = H - K + 1  # 244
CH = 64
C3 = K * K  # 169
NL = 128  # conv3 lo output channels (full M)
NH = C3 - NL  # 41 hi channels, DVE-folded into lo before the staircase
S_STRIP = 16
NC_ = 512  # chunk positions (2 image rows)
import os
PIPE = os.environ.get("K_PIPE", "1") == "1"      # stair deferral depth
DUP_GP = os.environ.get("K_DUP_GP", "1") == "1"  # dup DMAs on gpsimd queue


def _ap(t, off, dims):
    return bass.AP(t, off, [list(d) for d in dims])


def _mm(nc, out, lhsT, rhs, start, stop):
    nc.tensor.matmul(out, lhsT, rhs, start=start, stop=stop)


def build_nc():
    nc = bacc.Bacc("TRN2", target_bir_lowering=False, debug=False)

    x = nc.dram_tensor("x", [1, 1, H, W], F32, kind="ExternalInput")
    w1 = nc.dram_tensor("W1", [CH, 1, 3, 3], F32, kind="ExternalInput")
    b1 = nc.dram_tensor("b1", [CH], F32, kind="ExternalInput")
    w2 = nc.dram_tensor("W2", [CH, CH, 3, 3], F32, kind="ExternalInput")
    b2 = nc.dram_tensor("b2", [CH], F32, kind="ExternalInput")
    w3 = nc.dram_tensor("W3", [C3, CH, 3, 3], F32, kind="ExternalInput")
    b3 = nc.dram_tensor("b3", [C3], F32, kind="ExternalInput")
    y = nc.dram_tensor("y", [1, 1, HO, HO], F32, kind="ExternalOutput")
    xr = nc.dram_tensor("x_r", [H * W], F32R)

    with tile.TileContext(nc) as tc:
        Body(nc, tc, x, w1, b1, w2, b2, w3, b3, y, xr).build()

    nc.compile()
    return nc


class Body:
    def __init__(self, nc, tc, x, w1, b1, w2, b2, w3, b3, y, xr):
        self.nc, self.tc = nc, tc
        self.x, self.w1, self.b1, self.w2, self.b2 = x, w1, b1, w2, b2
        self.w3, self.b3, self.y, self.xr = w3, b3, y, xr

    def build(self):
        nc, tc = self.nc, self.tc
        with (
            tc.tile_pool(name="consts", bufs=1) as consts,
            tc.tile_pool(name="tlo", bufs=4) as p_tlo,
            tc.tile_pool(name="thi", bufs=2) as p_thi,
            tc.tile_pool(name="imc", bufs=7) as p_imc,
            tc.tile_pool(name="h1p", bufs=2) as p_h1,
            tc.tile_pool(name="h2p", bufs=2) as p_h2,
            tc.tile_pool(name="xsl", bufs=3) as p_xsl,
            tc.tile_pool(name="xsh", bufs=3) as p_xsh,
            tc.tile_pool(name="yout", bufs=1) as p_y,
        ):
            self.consts = consts
            self.p_tlo, self.p_thi, self.p_imc = p_tlo, p_thi, p_imc
            self.p_h1, self.p_h2, self.p_xsl, self.p_xsh = p_h1, p_h2, p_xsl, p_xsh
            self.p_y = p_y
            self._build_consts()  # uses (and releases) a 1-bank PSUM pool
            self.ps12 = tc.alloc_tile_pool(name="ps12", bufs=3, space="PSUM")
            self.ps3 = tc.alloc_tile_pool(name="ps3", bufs=2, space="PSUM")
            self.psy = tc.alloc_tile_pool(name="psy", bufs=1, space="PSUM")
            self._build_strips()
            self.psy.release()
            self.ps3.release()
            self.ps12.release()

    def _build_consts(self):
        nc, tc, consts = self.nc, self.tc, self.consts
        stage = tc.alloc_tile_pool(name="stage", bufs=1)
        pwtr = tc.alloc_tile_pool(name="wtr", bufs=1, space="PSUM")

        # Weights arrive [co, ci, du, dv]; matmuls need [ci, co] per tap.
        # A strided gather DMA would be 4-byte-descriptor-bound, so load
        # contiguously and transpose on the PE instead.
        from concourse.masks import make_identity

        ident = stage.tile([128, 128], F32)
        make_identity(nc, ident[:])

        w1raw = stage.tile([CH, 9], F32)
        nc.sync.dma_start(out=w1raw[:], in_=_ap(self.w1, 0, [(9, CH), (1, 9)]))
        w2raw = stage.tile([CH, 9 * CH], F32)
        nc.sync.dma_start(out=w2raw[:], in_=_ap(self.w2, 0, [(9 * CH, CH), (1, 9 * CH)]))
        w3raw_a = stage.tile([NL, 9 * CH], F32)
        nc.sync.dma_start(
            out=w3raw_a[:], in_=_ap(self.w3, 0, [(9 * CH, NL), (1, 9 * CH)])
        )
        w3raw_b = stage.tile([NH, 9 * CH], F32)
        nc.sync.dma_start(
            out=w3raw_b[:],
            in_=_ap(self.w3, NL * 9 * CH, [(9 * CH, NH), (1, 9 * CH)]),
        )

        def tapv(raw, t, n):  # [n_co, ci] view of tap t
            return raw[0:n, :].rearrange("p (ci t) -> p t ci", t=9)[:, t, :]

        # w1: lhsT [9 taps, 64 co]
        pT = pwtr.tile([128, 128], F32, tag="wtr")
        nc.tensor.transpose(pT[0:9, 0:CH], w1raw[:], ident[0:CH, 0:CH])
        self.w1sb = consts.tile([9, CH], F32R)
        nc.vector.tensor_copy(self.w1sb[:], pT[0:9, 0:CH])

        # Transpose each tap to PSUM base 0 (HW requires base 0), cast to
        # fp32r; upper (tap (du,1)) halves staged then partition-shifted to
        # partitions 64-127 by one SBUF->SBUF DMA per weight tile.
        self.w2p = consts.tile([2 * CH, 3 * CH], F32R)
        self.w2s = consts.tile([CH, 3 * CH], F32R)
        self.w3p_lo = consts.tile([2 * CH, 3 * NL], F32R)
        self.w3p_hi = consts.tile([2 * CH, 3 * NH], F32R)
        self.w3s_lo = consts.tile([CH, 3 * NL], F32R)
        self.w3s_hi = consts.tile([CH, 3 * NH], F32R)
        w2pu = stage.tile([CH, 3 * CH], F32R)
        w3pu_lo = stage.tile([CH, 3 * NL], F32R)
        w3pu_hi = stage.tile([CH, 3 * NH], F32R)

        def tr(dst, raw, t, n):
            pT = pwtr.tile([CH, 128], F32, tag="wtr")
            nc.tensor.transpose(pT[:, 0:n], tapv(raw, t, n), ident[0:n, 0:n])
            nc.vector.tensor_copy(dst, pT[:, 0:n])

        for p in range(3):
            cw = slice(p * CH, (p + 1) * CH)
            cl = slice(p * NL, (p + 1) * NL)
            ch = slice(p * NH, (p + 1) * NH)
            tr(self.w2p[0:CH, cw], w2raw, p * 3, CH)
            tr(w2pu[:, cw], w2raw, p * 3 + 1, CH)
            tr(self.w2s[:, cw], w2raw, p * 3 + 2, CH)
            tr(self.w3p_lo[0:CH, cl], w3raw_a, p * 3, NL)
            tr(w3pu_lo[:, cl], w3raw_a, p * 3 + 1, NL)
            tr(self.w3s_lo[:, cl], w3raw_a, p * 3 + 2, NL)
            tr(self.w3p_hi[0:CH, ch], w3raw_b, p * 3, NH)
            tr(w3pu_hi[:, ch], w3raw_b, p * 3 + 1, NH)
            tr(self.w3s_hi[:, ch], w3raw_b, p * 3 + 2, NH)
        nc.sync.dma_start(out=self.w2p[CH:, :], in_=w2pu[:])
        nc.sync.dma_start(out=self.w3p_lo[CH:, :], in_=w3pu_lo[:])
        nc.sync.dma_start(out=self.w3p_hi[CH:, :], in_=w3pu_hi[:])

        self.b1sb = consts.tile([CH, 1], F32)
        nc.scalar.dma_start(out=self.b1sb[:], in_=_ap(self.b1, 0, [(1, CH), (0, 1)]))
        self.b2sb = consts.tile([CH, 1], F32)
        nc.scalar.dma_start(out=self.b2sb[:], in_=_ap(self.b2, 0, [(1, CH), (0, 1)]))
        self.b3lo = consts.tile([NL, 1], F32)
        nc.scalar.dma_start(out=self.b3lo[:], in_=_ap(self.b3, 0, [(1, NL), (0, 1)]))
        self.b3hi = consts.tile([NH, 1], F32)
        nc.scalar.dma_start(out=self.b3hi[:], in_=_ap(self.b3, NL, [(1, NH), (0, 1)]))

        # staircase-ones: stair[:, 128] = 1, else 0; column j of the view
        # stair[:, 128-j : 256-j] is all-ones -> matmul writes the partition
        # sum into PSUM row j (zeros elsewhere, harmless under accumulation)
        stair_st = stage.tile([NL, 256], F32)
        nc.vector.memset(stair_st[:], 0.0)
        nc.vector.memset(stair_st[:, 128:129], 1.0)
        self.stair = consts.tile([NL, 256], F32R)
        nc.vector.tensor_copy(self.stair[:], stair_st[:])

        # fp32r zeros for h-tile junk tails (memset can't encode fp32r)
        self.zs = consts.tile([2 * CH, 772], F32R)
        nc.vector.memset(self.zs[:].bitcast(F32), 0.0)

        # x -> fp32r copy in DRAM (source of conv1 im2col chunks)
        xst = stage.tile([128, H * W // 128], F32)
        nc.sync.dma_start(
            out=xst[:], in_=_ap(self.x, 0, [(H * W // 128, 128), (1, H * W // 128)])
        )
        xsr = stage.tile([128, H * W // 128], F32R)
        nc.vector.tensor_copy(xsr[:], xst[:])
        nc.sync.dma_start(
            out=_ap(self.xr, 0, [(H * W // 128, 128), (1, H * W // 128)]), in_=xsr[:]
        )
        pwtr.release()
        stage.release()

    # ---------------- per-strip stages ----------------

    def emit_conv1_dmas(self, i0, S):
        """Issue the im2col DMAs for a whole strip, prefetched one strip
        ahead of the matmuls that consume them."""
        nc = self.nc
        c0 = i0 + 6
        L1 = (S + 6) * W
        tiles = []
        for hs in range(0, L1, 2 * NC_):
            he = min(hs + 2 * NC_, L1)
            imc = self.p_imc.tile([9, 2 * NC_], F32R, tag="imc")
            nc.sync.dma_start(
                out=imc[:, 0 : he - hs],
                in_=_ap(self.xr, (c0 - 5) * W - 1 + hs, [(W, 3), (1, 3), (1, he - hs)]),
            )
            tiles.append((hs, he, imc))
        return (i0, S, tiles)

    def _conv1_start(self, pend):
        i0, S, tiles = pend
        nc = self.nc
        L1 = (S + 6) * W
        h1t = self.p_h1.tile([2 * CH, (S_STRIP + 6) * W + 772], F32R, tag="h1")
        nc.vector.tensor_copy(h1t[0:CH, L1 : L1 + 772], self.zs[0:CH])
        nc.vector.tensor_copy(h1t[CH:, L1 - 1 : L1 + 771], self.zs[CH:])
        chunks = []
        for hs, he, imc in tiles:
            for cs in range(hs, he, NC_):
                chunks.append((cs, min(cs + NC_, L1), imc, hs))
        return {"h1t": h1t, "L1": L1, "Lh": (L1 // (2 * NC_)) * NC_,
                "chunks": chunks, "next": 0}

    def _conv1_emit_upto(self, st, n):
        """Emit conv1 chunk matmuls (+relu +dup) until `n` chunks done."""
        nc = self.nc
        h1t, L1, Lh = st["h1t"], st["L1"], st["Lh"]
        while st["next"] < min(n, len(st["chunks"])):
            cs, ce, imc, hs = st["chunks"][st["next"]]
            st["next"] += 1
            pt = self.ps12.tile([CH, NC_], F32, tag="ps12")
            _mm(nc, pt[:, 0 : ce - cs], self.w1sb[:], imc[:, cs - hs : ce - hs],
                True, True)
            nc.scalar.activation(
                h1t[0:CH, cs:ce], pt[:, 0 : ce - cs], AF.Relu, bias=self.b1sb[:]
            )
            # half-strip shifted dup: upper[q] = lower[q+1]
            dup = nc.gpsimd if DUP_GP else nc.sync
            if ce == Lh:
                dup.dma_start(out=h1t[CH:, 0 : Lh - 1], in_=h1t[0:CH, 1:Lh])
            elif ce == L1:
                dup.dma_start(out=h1t[CH:, Lh - 1 : L1 - 1], in_=h1t[0:CH, Lh:L1])

    def emit_xs_pair(self, i0, r2):
        """xs patch tiles for one chunk-PAIR (4 image rows). Channel
        c=(u*13+v) needs x[i0+r2+u+s, v:v+HO] for s=0..3; as a flat read
        from x[(i0+r2+u)*W + v] those four windows live at col offsets
        s*W inside one contiguous 1012-float span, so ONE 3-dim DMA per
        partition block covers both chunks (trigger cost ~700ns/DMA on
        the issuing queue engine, so fewer+bigger wins)."""
        nc = self.nc
        SPAN = 3 * W + HO  # 1012
        r = (i0 + r2) * W
        xs_lo = self.p_xsl.tile([NL, 4 * W], F32, tag="xsl")
        xs_hi = self.p_xsh.tile([NH, 4 * W], F32, tag="xsh")
        nc.sync.dma_start(
            out=xs_lo[0:117, 0:SPAN], in_=_ap(self.x, r, [(W, 9), (1, K), (1, SPAN)])
        )
        nc.sync.dma_start(
            out=xs_lo[117:128, 0:SPAN],
            in_=_ap(self.x, r + 9 * W, [(1, 11), (1, SPAN)]),
        )
        nc.gpsimd.dma_start(
            out=xs_hi[0:2, 0:SPAN],
            in_=_ap(self.x, r + 9 * W + 11, [(1, 2), (1, SPAN)]),
        )
        nc.gpsimd.dma_start(
            out=xs_hi[2:41, 0:SPAN],
            in_=_ap(self.x, r + 10 * W, [(W, 3), (1, K), (1, SPAN)]),
        )
        return xs_lo, xs_hi

    def emit_conv2(self, i0, S, h1t):
        nc = self.nc
        L2 = (S + 3) * W
        h2t = self.p_h2.tile([2 * CH, (S_STRIP + 3) * W + 772], F32R, tag="h2")
        nc.vector.tensor_copy(h2t[0:CH, L2 : L2 + 772], self.zs[0:CH])
        nc.vector.tensor_copy(h2t[CH:, L2 - 1 : L2 + 771], self.zs[CH:])
        Lh = (L2 // (2 * NC_)) * NC_
        # two chunks' accumulation chains interleaved: consecutive matmuls
        # then target ALTERNATING PSUM banks, so drains overlap (same-bank
        # back-to-back matmuls serialize their drain, ~280ns vs 213ns)
        chunks = [(cs, min(cs + NC_, L2)) for cs in range(0, L2, NC_)]
        for k in range(0, len(chunks), 2):
            grp = []
            for cs, ce in chunks[k : k + 2]:
                pt = self.ps12.tile([CH, NC_], F32, tag="ps12")
                grp.append((pt, cs, ce))
            for p in range(3):
                off = p * W + 255
                for pt, cs, ce in grp:
                    _mm(nc, pt[:, 0 : ce - cs], self.w2p[:, p * CH : (p + 1) * CH],
                        h1t[:, cs + off : ce + off], p == 0, False)
                if k == 0 and p == 0:
                    # drain the previous strip's deferred staircase matmuls
                    # here (their DVE products are ready by now) instead of
                    # stalling the PE on the DVE at the strip boundary
                    self.flush_stair(keep=0)
            for p in range(3):
                off = p * W + 2 + 255
                for pt, cs, ce in grp:
                    _mm(nc, pt[:, 0 : ce - cs], self.w2s[:, p * CH : (p + 1) * CH],
                        h1t[0:CH, cs + off : ce + off], False, p == 2)
            for pt, cs, ce in grp:
                nc.scalar.activation(
                    h2t[0:CH, cs:ce], pt[:, 0 : ce - cs], AF.Relu, bias=self.b2sb[:]
                )
                dup = nc.gpsimd if DUP_GP else nc.sync
                if ce == Lh:
                    dup.dma_start(out=h2t[CH:, 0 : Lh - 1], in_=h2t[0:CH, 1:Lh])
                elif ce == L2:
                    dup.dma_start(out=h2t[CH:, Lh - 1 : L2 - 1], in_=h2t[0:CH, Lh:L2])
        return h2t

    def emit_conv3_chunk(self, i0, cs, h2t, xs_lo, xs_hi):
        """conv3 + stt + fold for one 2-row chunk; the staircase matmul is
        deferred (pend) so the PE never waits on the DVE."""
        nc = self.nc
        r2 = cs // W
        par = (r2 // 2) % 2  # chunk's slot within its xs pair tile
        plo = self.ps3.tile([NL, NC_], F32, tag="ps3lo")
        phi = self.ps3.tile([NH, NC_], F32, tag="ps3hi")
        for p in range(3):
            off = p * W + 255
            _mm(nc, plo[:], self.w3p_lo[:, p * NL : (p + 1) * NL],
                h2t[:, cs + off : cs + NC_ + off], p == 0, False)
            _mm(nc, phi[:], self.w3p_hi[:, p * NH : (p + 1) * NH],
                h2t[:, cs + off : cs + NC_ + off], p == 0, False)
        for p in range(3):
            off = p * W + 2 + 255
            _mm(nc, plo[:], self.w3s_lo[:, p * NL : (p + 1) * NL],
                h2t[0:CH, cs + off : cs + NC_ + off], False, p == 2)
            _mm(nc, phi[:], self.w3s_hi[:, p * NH : (p + 1) * NH],
                h2t[0:CH, cs + off : cs + NC_ + off], False, p == 2)
        jj = (i0 + r2) // 2
        # t = (conv3_psum + b3) * xs, fp32r, straight from PSUM on the DVE
        t_lo = self.p_tlo.tile([NL, 2 * HO], F32R, tag="tlo")
        t_hi = self.p_thi.tile([NH, 2 * HO], F32R, tag="thi")
        for ps_t, xs_t, t_t, b3_t in (
            (plo, xs_lo, t_lo, self.b3lo),
            (phi, xs_hi, t_hi, self.b3hi),
        ):
            wv = ps_t[:].rearrange("p (r c) -> p r c", c=W)[:, :, 6 : 6 + HO]
            xv = xs_t[:].rearrange("p (r c) -> p r c", c=W)[
                :, 2 * par : 2 * par + 2, 0:HO
            ]
            tv = t_t[:].rearrange("p (r c) -> p r c", c=HO)
            nc.vector.scalar_tensor_tensor(
                out=tv, in0=wv, scalar=b3_t[:], in1=xv, op0=ALU.add, op1=ALU.mult
            )
        # fold the 41 hi channels into lo so one K=128 staircase mm suffices
        # (on gpsimd: keeps the DVE queue's stt cadence short, since the
        # stts gate conv3's PSUM bank recycling)
        nc.gpsimd.tensor_tensor(
            out=t_lo[0:NH, :], in0=t_lo[0:NH, :], in1=t_hi[:], op=ALU.add
        )
        self.pend.append((t_lo, jj))

    def flush_stair(self, keep=0):
        nc = self.nc
        while len(self.pend) > keep:
            t_lo, jj = self.pend.pop(0)
            stop = jj == self.NYC - 1
            _mm(nc, self.psum_y[:], self.stair[:, 128 - jj : 256 - jj],
                t_lo[:], self.first_mm, stop)
            self.first_mm = False

    def _build_strips(self):
        nc = self.nc
        self.NYC = (HO * HO) // 488  # 122
        self.psum_y = self.psy.tile([128, 488], F32)
        self.pend = []
        self.first_mm = True

        strips = []
        i0 = 0
        while i0 < HO:
            strips.append((i0, min(S_STRIP, HO - i0)))
            i0 += S_STRIP

        st = self._conv1_start(self.emit_conv1_dmas(*strips[0]))
        self._conv1_emit_upto(st, len(st["chunks"]))
        h1t = st["h1t"]
        for si, (i0, S) in enumerate(strips):
            pend_c1 = (
                self.emit_conv1_dmas(*strips[si + 1])
                if si + 1 < len(strips)
                else None
            )
            h2t = self.emit_conv2(i0, S, h1t)
            c1 = self._conv1_start(pend_c1) if pend_c1 else None
            xs = None
            for ci, cs in enumerate(range(0, S * W, NC_)):
                # front-load next strip's conv1: 2 chunks per conv3 chunk
                # (done by mid-phase) so its relu tail never delays the
                # next conv2 start; single matmuls sprinkled into dense
                # stretches keep the PE busy enough for HAM to stay warm
                if c1 is not None:
                    self._conv1_emit_upto(c1, 2 * ci + 1)
                if ci % 2 == 0:
                    xs = self.emit_xs_pair(i0, cs // W)
                self.emit_conv3_chunk(i0, cs, h2t, *xs)
                self.flush_stair(keep=3 if PIPE else 0)
                if c1 is not None:
                    self._conv1_emit_upto(c1, 2 * ci + 2)
            if c1 is not None:
                h1t = c1["h1t"]
        self.flush_stair(keep=0)

        ysb = self.p_y.tile([self.NYC, 488], F32)
        nc.vector.tensor_copy(ysb[:], self.psum_y[0 : self.NYC, :])
        nc.sync.dma_start(
            out=_ap(self.y, 0, [(488, self.NYC), (1, 488)]), in_=ysb[:]
        )


_NC_CACHE = {}


def _get_nc():
    if "nc" not in _NC_CACHE:
        _NC_CACHE["nc"] = build_nc()
    return _NC_CACHE["nc"]


def _in_maps(inputs):
    x = np.ascontiguousarray(np.asarray(inputs["x"], dtype=np.float32))
    names = ["W1", "b1", "W2", "b2", "W3", "b3"]
    ws = {n: np.ascontiguousarray(np.asarray(inputs[n], np.float32)) for n in names}
    maps = []
    for i in range(8):
        m = {"x": x[i : i + 1]}
        m.update(ws)
        maps.append(m)
    return maps


def kernel(**inputs):
    nc = _get_nc()
    res = run_bass_kernel_spmd(nc, _in_maps(inputs), list(range(8)))
    return np.concatenate([res.results[i]["y"] for i in range(8)], axis=0)


def profile(**inputs):
    nc = _get_nc()
    res = run_bass_kernel_spmd(nc, _in_maps(inputs), list(range(8)), trace=True)
    return res.exec_time_ns


if __name__ == "__main__":
    rng = np.random.RandomState(0)
    ins = {
        "x": rng.randn(8, 1, H, W).astype(np.float32),
        "W1": rng.randn(CH, 1, 3, 3).astype(np.float32) * 0.1,
        "b1": np.zeros(CH, np.float32),
        "W2": rng.randn(CH, CH, 3, 3).astype(np.float32) * 0.05,
        "b2": np.zeros(CH, np.float32),
        "W3": rng.randn(C3, CH, 3, 3).astype(np.float32) * 0.05,
        "b3": np.zeros(C3, np.float32),
    }
    print(kernel(**ins).shape)


# revision 25
# speedup vs baseline: 1.0019x; 1.0019x over previous
"""NlmCNN (weight-predicting CNN + per-pixel 13x13 weighted sum) on 8 trn2 cores.

Sharding: data-parallel over batch (8 images -> 8 cores), weights replicated.

Per-core layout trick: output y is the conv stack's result center-cropped by
6 pixels, and the receptive field of the three 3x3 convs only reaches 3 px
out, so SAME-padding never materializes: every layer is computed VALID-style
on an unpadded 256-stride flat layout. Column-wrap junk from flat shifted
reads stays confined to the outer <=3 columns of each layer, which are
discarded by the crop.

Pipeline per strip of S output rows:
  conv1: per-2-chunk im2col [9, 1024] via one 3-dim DMA -> K=9 matmul.
         DMAs for strip i+1 are issued at the START of strip i (so the
         transfers never race the strip-boundary traffic); the matmuls
         are interleaved 1-2 at a time into strip i's conv3 phase so the
         PE never sits in an ACT-throughput-bound conv1 burst (which
         would trip the HAM throttle down to half clock).
  conv2/conv3: taps (du,0)+(du,1) fused into K=128 matmuls against an SBUF
         tile whose upper 64 partitions hold h shifted by +1 (built by two
         SBUF->SBUF SWDGE DMAs per strip); taps (du,2) are K=64 singles.
         conv3's 169 output channels split 128+41 (full-M matmuls first).
  einsum: patch matrix xs[c, s] = x[pos + shift(c)] via per-chunk strided
         DMAs (6-deep pool, so transfers spread across the strip);
         DVE scalar_tensor_tensor computes t = (conv3_psum + b3) * xs from
         PSUM; the 41 hi channels are DVE-folded into the lo tile, then a
         single K=128 "staircase ones" matmul per chunk (deferred so the
         PE never waits on the DVE) reduces over partitions, accumulating
         2-row chunk j into row j of one persistent PSUM tile; one copy +
         one DMA store the whole image.

All matmul inputs are float32r (fp32 streamed at bf16 rate, ~1e-4 relerr).
"""

import numpy as np

import concourse.bacc as bacc
import concourse.bass as bass
import concourse.mybir as mybir
import concourse.tile as tile
from concourse.bass_utils import run_bass_kernel_spmd

F32 = mybir.dt.float32
F32R = mybir.dt.float32r
AF = mybir.ActivationFunctionType
ALU = mybir.AluOpType

H = 256
W = 256
K = 13
HO = H - K + 1  # 244
CH = 64
C3 = K * K  # 169
NL = 128  # conv3 lo output channels (full M)
NH = C3 - NL  # 41 hi channels, DVE-folded into lo before the staircase
S_STRIP = 16
NC_ = 512  # chunk positions (2 image rows)
import os
PIPE = os.environ.get("K_PIPE", "1") == "1"      # stair deferral depth
DUP_GP = os.environ.get("K_DUP_GP", "1") == "1"  # dup DMAs on gpsimd queue


def _ap(t, off, dims):
    return bass.AP(t, off, [list(d) for d in dims])


def _mm(nc, out, lhsT, rhs, start, stop):
    nc.tensor.matmul(out, lhsT, rhs, start=start, stop=stop)


def build_nc():
    nc = bacc.Bacc("TRN2", target_bir_lowering=False, debug=False)

    x = nc.dram_tensor("x", [1, 1, H, W], F32, kind="ExternalInput")
    w1 = nc.dram_tensor("W1", [CH, 1, 3, 3], F32, kind="ExternalInput")
    b1 = nc.dram_tensor("b1", [CH], F32, kind="ExternalInput")
    w2 = nc.dram_tensor("W2", [CH, CH, 3, 3], F32, kind="ExternalInput")
    b2 = nc.dram_tensor("b2", [CH], F32, kind="ExternalInput")
    w3 = nc.dram_tensor("W3", [C3, CH, 3, 3], F32, kind="ExternalInput")
    b3 = nc.dram_tensor("b3", [C3], F32, kind="ExternalInput")
    y = nc.dram_tensor("y", [1, 1, HO, HO], F32, kind="ExternalOutput")
    xr = nc.dram_tensor("x_r", [H * W], F32R)

    with tile.TileContext(nc) as tc:
        Body(nc, tc, x, w1, b1, w2, b2, w3, b3, y, xr).build()

    nc.compile()
    return nc


class Body:
    def __init__(self, nc, tc, x, w1, b1, w2, b2, w3, b3, y, xr):
        self.nc, self.tc = nc, tc
        self.x, self.w1, self.b1, self.w2, self.b2 = x, w1, b1, w2, b2
        self.w3, self.b3, self.y, self.xr = w3, b3, y, xr

    def build(self):
        nc, tc = self.nc, self.tc
        with (
            tc.tile_pool(name="consts", bufs=1) as consts,
            tc.tile_pool(name="tlo", bufs=4) as p_tlo,
            tc.tile_pool(name="thi", bufs=2) as p_thi,
            tc.tile_pool(name="imc", bufs=7) as p_imc,
            tc.tile_pool(name="h1p", bufs=2) as p_h1,
            tc.tile_pool(name="h2p", bufs=2) as p_h2,
            tc.tile_pool(name="xsl", bufs=3) as p_xsl,
            tc.tile_pool(name="xsh", bufs=3) as p_xsh,
            tc.tile_pool(name="yout", bufs=1) as p_y,
        ):
            self.consts = consts
            self.p_tlo, self.p_thi, self.p_imc = p_tlo, p_thi, p_imc
            self.p_h1, self.p_h2, self.p_xsl, self.p_xsh = p_h1, p_h2, p_xsl, p_xsh
            self.p_y = p_y
            self._build_consts()  # uses (and releases) a 1-bank PSUM pool
            self.ps12 = tc.alloc_tile_pool(name="ps12", bufs=3, space="PSUM")
            self.ps3 = tc.alloc_tile_pool(name="ps3", bufs=2, space="PSUM")
            self.psy = tc.alloc_tile_pool(name="psy", bufs=1, space="PSUM")
            self._build_strips()
            self.psy.release()
            self.ps3.release()
            self.ps12.release()

    def _build_consts(self):
        nc, tc, consts = self.nc, self.tc, self.consts
        stage = tc.alloc_tile_pool(name="stage", bufs=1)
        pwtr = tc.alloc_tile_pool(name="wtr", bufs=1, space="PSUM")

        # Weights arrive [co, ci, du, dv]; matmuls need [ci, co] per tap.
        # A strided gather DMA would be 4-byte-descriptor-bound, so load
        # contiguously and transpose on the PE instead.
        from concourse.masks import make_identity

        ident = stage.tile([128, 128], F32)
        make_identity(nc, ident[:])

        w1raw = stage.tile([CH, 9], F32)
        nc.sync.dma_start(out=w1raw[:], in_=_ap(self.w1, 0, [(9, CH), (1, 9)]))
        w2raw = stage.tile([CH, 9 * CH], F32)
        nc.sync.dma_start(out=w2raw[:], in_=_ap(self.w2, 0, [(9 * CH, CH), (1, 9 * CH)]))
        w3raw_a = stage.tile([NL, 9 * CH], F32)
        nc.sync.dma_start(
            out=w3raw_a[:], in_=_ap(self.w3, 0, [(9 * CH, NL), (1, 9 * CH)])
        )
        w3raw_b = stage.tile([NH, 9 * CH], F32)
        nc.sync.dma_start(
            out=w3raw_b[:],
            in_=_ap(self.w3, NL * 9 * CH, [(9 * CH, NH), (1, 9 * CH)]),
        )

        def tapv(raw, t, n):  # [n_co, ci] view of tap t
            return raw[0:n, :].rearrange("p (ci t) -> p t ci", t=9)[:, t, :]

        # w1: lhsT [9 taps, 64 co]
        pT = pwtr.tile([128, 128], F32, tag="wtr")
        nc.tensor.transpose(pT[0:9, 0:CH], w1raw[:], ident[0:CH, 0:CH])
        self.w1sb = consts.tile([9, CH], F32R)
        nc.vector.tensor_copy(self.w1sb[:], pT[0:9, 0:CH])

        # Transpose each tap to PSUM base 0 (HW requires base 0), cast to
        # fp32r; upper (tap (du,1)) halves staged then partition-shifted to
        # partitions 64-127 by one SBUF->SBUF DMA per weight tile.
        self.w2p = consts.tile([2 * CH, 3 * CH], F32R)
        self.w2s = consts.tile([CH, 3 * CH], F32R)
        self.w3p_lo = consts.tile([2 * CH, 3 * NL], F32R)
        self.w3p_hi = consts.tile([2 * CH, 3 * NH], F32R)
        self.w3s_lo = consts.tile([CH, 3 * NL], F32R)
        self.w3s_hi = consts.tile([CH, 3 * NH], F32R)
        w2pu = stage.tile([CH, 3 * CH], F32R)
        w3pu_lo = stage.tile([CH, 3 * NL], F32R)
        w3pu_hi = stage.tile([CH, 3 * NH], F32R)

        def tr(dst, raw, t, n):
            pT = pwtr.tile([CH, 128], F32, tag="wtr")
            nc.tensor.transpose(pT[:, 0:n], tapv(raw, t, n), ident[0:n, 0:n])
            nc.vector.tensor_copy(dst, pT[:, 0:n])

        for p in range(3):
            cw = slice(p * CH, (p + 1) * CH)
            cl = slice(p * NL, (p + 1) * NL)
            ch = slice(p * NH, (p + 1) * NH)
            tr(self.w2p[0:CH, cw], w2raw, p * 3, CH)
            tr(w2pu[:, cw], w2raw, p * 3 + 1, CH)
            tr(self.w2s[:, cw], w2raw, p * 3 + 2, CH)
            tr(self.w3p_lo[0:CH, cl], w3raw_a, p * 3, NL)
            tr(w3pu_lo[:, cl], w3raw_a, p * 3 + 1, NL)
            tr(self.w3s_lo[:, cl], w3raw_a, p * 3 + 2, NL)
            tr(self.w3p_hi[0:CH, ch], w3raw_b, p * 3, NH)
            tr(w3pu_hi[:, ch], w3raw_b, p * 3 + 1, NH)
            tr(self.w3s_hi[:, ch], w3raw_b, p * 3 + 2, NH)
        nc.sync.dma_start(out=self.w2p[CH:, :], in_=w2pu[:])
        nc.sync.dma_start(out=self.w3p_lo[CH:, :], in_=w3pu_lo[:])
        nc.sync.dma_start(out=self.w3p_hi[CH:, :], in_=w3pu_hi[:])

        self.b1sb = consts.tile([CH, 1], F32)
        nc.scalar.dma_start(out=self.b1sb[:], in_=_ap(self.b1, 0, [(1, CH), (0, 1)]))
        self.b2sb = consts.tile([CH, 1], F32)
        nc.scalar.dma_start(out=self.b2sb[:], in_=_ap(self.b2, 0, [(1, CH), (0, 1)]))
        self.b3lo = consts.tile([NL, 1], F32)
        nc.scalar.dma_start(out=self.b3lo[:], in_=_ap(self.b3, 0, [(1, NL), (0, 1)]))
        self.b3hi = consts.tile([NH, 1], F32)
        nc.scalar.dma_start(out=self.b3hi[:], in_=_ap(self.b3, NL, [(1, NH), (0, 1)]))

        # staircase-ones: stair[:, 128] = 1, else 0; column j of the view
        # stair[:, 128-j : 256-j] is all-ones -> matmul writes the partition
        # sum into PSUM row j (zeros elsewhere, harmless under accumulation)
        stair_st = stage.tile([NL, 256], F32)
        nc.vector.memset(stair_st[:], 0.0)
        nc.vector.memset(stair_st[:, 128:129], 1.0)
        self.stair = consts.tile([NL, 256], F32R)
        nc.vector.tensor_copy(self.stair[:], stair_st[:])

        # fp32r zeros for h-tile junk tails (memset can't encode fp32r)
        self.zs = consts.tile([2 * CH, 772], F32R)
        nc.vector.memset(self.zs[:].bitcast(F32), 0.0)

        # x -> fp32r copy in DRAM (source of conv1 im2col chunks)
        xst = stage.tile([128, H * W // 128], F32)
        nc.sync.dma_start(
            out=xst[:], in_=_ap(self.x, 0, [(H * W // 128, 128), (1, H * W // 128)])
        )
        xsr = stage.tile([128, H * W // 128], F32R)
        nc.vector.tensor_copy(xsr[:], xst[:])
        nc.sync.dma_start(
            out=_ap(self.xr, 0, [(H * W // 128, 128), (1, H * W // 128)]), in_=xsr[:]
        )
        pwtr.release()
        stage.release()

    # ---------------- per-strip stages ----------------

    def emit_conv1_dmas(self, i0, S):
        """Issue the im2col DMAs for a whole strip, prefetched one strip
        ahead of the matmuls that consume them."""
        nc = self.nc
        c0 = i0 + 6
        L1 = (S + 6) * W
        tiles = []
        for hs in range(0, L1, 2 * NC_):
            he = min(hs + 2 * NC_, L1)
            imc = self.p_imc.tile([9, 2 * NC_], F32R, tag="imc")
            nc.sync.dma_start(
                out=imc[:, 0 : he - hs],
                in_=_ap(self.xr, (c0 - 5) * W - 1 + hs, [(W, 3), (1, 3), (1, he - hs)]),
            )
            tiles.append((hs, he, imc))
        return (i0, S, tiles)

    def _conv1_start(self, pend):
        i0, S, tiles = pend
        nc = self.nc
        L1 = (S + 6) * W
        h1t = self.p_h1.tile([2 * CH, (S_STRIP + 6) * W + 772], F32R, tag="h1")
        nc.vector.tensor_copy(h1t[0:CH, L1 : L1 + 772], self.zs[0:CH])
        nc.vector.tensor_copy(h1t[CH:, L1 - 1 : L1 + 771], self.zs[CH:])
        chunks = []
        for hs, he, imc in tiles:
            for cs in range(hs, he, NC_):
                chunks.append((cs, min(cs + NC_, L1), imc, hs))
        return {"h1t": h1t, "L1": L1, "Lh": (L1 // (2 * NC_)) * NC_,
                "chunks": chunks, "next": 0}

    def _conv1_emit_upto(self, st, n):
        """Emit conv1 chunk matmuls (+relu +dup) until `n` chunks done."""
        nc = self.nc
        h1t, L1, Lh = st["h1t"], st["L1"], st["Lh"]
        while st["next"] < min(n, len(st["chunks"])):
            cs, ce, imc, hs = st["chunks"][st["next"]]
            st["next"] += 1
            pt = self.ps12.tile([CH, NC_], F32, tag="ps12")
            _mm(nc, pt[:, 0 : ce - cs], self.w1sb[:], imc[:, cs - hs : ce - hs],
                True, True)
            nc.scalar.activation(
                h1t[0:CH, cs:ce], pt[:, 0 : ce - cs], AF.Relu, bias=self.b1sb[:]
            )
            # half-strip shifted dup: upper[q] = lower[q+1]
            dup = nc.gpsimd if DUP_GP else nc.sync
            if ce == Lh:
                dup.dma_start(out=h1t[CH:, 0 : Lh - 1], in_=h1t[0:CH, 1:Lh])
            elif ce == L1:
                dup.dma_start(out=h1t[CH:, Lh - 1 : L1 - 1], in_=h1t[0:CH, Lh:L1])

    def emit_xs_pair(self, i0, r2):
        """xs patch tiles for one chunk-PAIR (4 image rows). Channel
        c=(u*13+v) needs x[i0+r2+u+s, v:v+HO] for s=0..3; as a flat read
        from x[(i0+r2+u)*W + v] those four windows live at col offsets
        s*W inside one contiguous 1012-float span, so ONE 3-dim DMA per
        partition block covers both chunks (trigger cost ~700ns/DMA on
        the issuing queue engine, so fewer+bigger wins)."""
        nc = self.nc
        SPAN = 3 * W + HO  # 1012
        r = (i0 + r2) * W
        xs_lo = self.p_xsl.tile([NL, 4 * W], F32, tag="xsl")
        xs_hi = self.p_xsh.tile([NH, 4 * W], F32, tag="xsh")
        nc.sync.dma_start(
            out=xs_lo[0:117, 0:SPAN], in_=_ap(self.x, r, [(W, 9), (1, K), (1, SPAN)])
        )
        nc.sync.dma_start(
            out=xs_lo[117:128, 0:SPAN],
            in_=_ap(self.x, r + 9 * W, [(1, 11), (1, SPAN)]),
        )
        nc.gpsimd.dma_start(
            out=xs_hi[0:2, 0:SPAN],
            in_=_ap(self.x, r + 9 * W + 11, [(1, 2), (1, SPAN)]),
        )
        nc.gpsimd.dma_start(
            out=xs_hi[2:41, 0:SPAN],
            in_=_ap(self.x, r + 10 * W, [(W, 3), (1, K), (1, SPAN)]),
        )
        return xs_lo, xs_hi

    def emit_conv2(self, i0, S, h1t):
        nc = self.nc
        L2 = (S + 3) * W
        h2t = self.p_h2.tile([2 * CH, (S_STRIP + 3) * W + 772], F32R, tag="h2")
        nc.vector.tensor_copy(h2t[0:CH, L2 : L2 + 772], self.zs[0:CH])
        nc.vector.tensor_copy(h2t[CH:, L2 - 1 : L2 + 771], self.zs[CH:])
        Lh = (L2 // (2 * NC_)) * NC_
        # two chunks' accumulation chains interleaved: consecutive matmuls
        # then target ALTERNATING PSUM banks, so drains overlap (same-bank
        # back-to-back matmuls serialize their drain, ~280ns vs 213ns)
        chunks = [(cs, min(cs + NC_, L2)) for cs in range(0, L2, NC_)]
        for k in range(0, len(chunks), 2):
            grp = []
            for cs, ce in chunks[k : k + 2]:
                pt = self.ps12.tile([CH, NC_], F32, tag="ps12")
                grp.append((pt, cs, ce))
            for p in range(3):
                off = p * W + 255
                for pt, cs, ce in grp:
                    _mm(nc, pt[:, 0 : ce - cs], self.w2p[:, p * CH : (p + 1) * CH],
                        h1t[:, cs + off : ce + off], p == 0, False)
                if k == 0 and p == 0:
                    # drain the previous strip's deferred staircase matmuls
                    # here (their DVE products are ready by now) instead of
                    # stalling the PE on the DVE at the strip boundary
                    self.flush_stair(keep=0)
            for p in range(3):
                off = p * W + 2 + 255
                for pt, cs, ce in grp:
                    _mm(nc, pt[:, 0 : ce - cs], self.w2s[:, p * CH : (p + 1) * CH],
                        h1t[0:CH, cs + off : ce + off], False, p == 2)
            for pt, cs, ce in grp:
                nc.scalar.activation(
                    h2t[0:CH, cs:ce], pt[:, 0 : ce - cs], AF.Relu, bias=self.b2sb[:]
                )
                dup = nc.gpsimd if DUP_GP else nc.sync
                if ce == Lh:
                    dup.dma_start(out=h2t[CH:, 0 : Lh - 1], in_=h2t[0:CH, 1:Lh])
                elif ce == L2:
                    dup.dma_start(out=h2t[CH:, Lh - 1 : L2 - 1], in_=h2t[0:CH, Lh:L2])
        return h2t

    def emit_conv3_chunk(self, i0, cs, h2t, xs_lo, xs_hi):
        """conv3 + stt + fold for one 2-row chunk; the staircase matmul is
        deferred (pend) so the PE never waits on the DVE."""
        nc = self.nc
        r2 = cs // W
        par = (r2 // 2) % 2  # chunk's slot within its xs pair tile
        plo = self.ps3.tile([NL, NC_], F32, tag="ps3lo")
        phi = self.ps3.tile([NH, NC_], F32, tag="ps3hi")
        for p in range(3):
            off = p * W + 255
            _mm(nc, plo[:], self.w3p_lo[:, p * NL : (p + 1) * NL],
                h2t[:, cs + off : cs + NC_ + off], p == 0, False)
            _mm(nc, phi[:], self.w3p_hi[:, p * NH : (p + 1) * NH],
                h2t[:, cs + off : cs + NC_ + off], p == 0, False)
        for p in range(3):
            off = p * W + 2 + 255
            _mm(nc, plo[:], self.w3s_lo[:, p * NL : (p + 1) * NL],
                h2t[0:CH, cs + off : cs + NC_ + off], False, p == 2)
            _mm(nc, phi[:], self.w3s_hi[:, p * NH : (p + 1) * NH],
                h2t[0:CH, cs + off : cs + NC_ + off], False, p == 2)
        jj = (i0 + r2) // 2
        # t = (conv3_psum + b3) * xs, fp32r, straight from PSUM on the DVE
        t_lo = self.p_tlo.tile([NL, 2 * HO], F32R, tag="tlo")
        t_hi = self.p_thi.tile([NH, 2 * HO], F32R, tag="thi")
        for ps_t, xs_t, t_t, b3_t in (
            (plo, xs_lo, t_lo, self.b3lo),
            (phi, xs_hi, t_hi, self.b3hi),
        ):
            wv = ps_t[:].rearrange("p (r c) -> p r c", c=W)[:, :, 6 : 6 + HO]
            xv = xs_t[:].rearrange("p (r c) -> p r c", c=W)[
                :, 2 * par : 2 * par + 2, 0:HO
            ]
            tv = t_t[:].rearrange("p (r c) -> p r c", c=HO)
            nc.vector.scalar_tensor_tensor(
                out=tv, in0=wv, scalar=b3_t[:], in1=xv, op0=ALU.add, op1=ALU.mult
            )
        # fold the 41 hi channels into lo so one K=128 staircase mm suffices
        # (on gpsimd: keeps the DVE queue's stt cadence short, since the
        # stts gate conv3's PSUM bank recycling)
        nc.gpsimd.tensor_tensor(
            out=t_lo[0:NH, :], in0=t_lo[0:NH, :], in1=t_hi[:], op=ALU.add
        )
        self.pend.append((t_lo, jj))

    def flush_stair(self, keep=0):
        nc = self.nc
        while len(self.pend) > keep:
            t_lo, jj = self.pend.pop(0)
            stop = jj == self.NYC - 1
            _mm(nc, self.psum_y[:], self.stair[:, 128 - jj : 256 - jj],
                t_lo[:], self.first_mm, stop)
            self.first_mm = False

    def _build_strips(self):
        nc = self.nc
        self.NYC = (HO * HO) // 488  # 122
        self.psum_y = self.psy.tile([128, 488], F32)
        self.pend = []
        self.first_mm = True

        strips = []
        i0 = 0
        while i0 < HO:
            strips.append((i0, min(S_STRIP, HO - i0)))
            i0 += S_STRIP

        st = self._conv1_start(self.emit_conv1_dmas(*strips[0]))
        self._conv1_emit_upto(st, len(st["chunks"]))
        h1t = st["h1t"]
        for si, (i0, S) in enumerate(strips):
            pend_c1 = (
                self.emit_conv1_dmas(*strips[si + 1])
                if si + 1 < len(strips)
                else None
            )
            h2t = self.emit_conv2(i0, S, h1t)
            c1 = self._conv1_start(pend_c1) if pend_c1 else None
            xs = None
            for ci, cs in enumerate(range(0, S * W, NC_)):
                # front-load next strip's conv1: 2 chunks per conv3 chunk
                # (done by mid-phase) so its relu tail never delays the
                # next conv2 start; single matmuls sprinkled into dense
                # stretches keep the PE busy enough for HAM to stay warm
                if c1 is not None:
                    self._conv1_emit_upto(c1, 2 * ci + 1)
                if ci % 2 == 0:
                    xs = self.emit_xs_pair(i0, cs // W)
                self.emit_conv3_chunk(i0, cs, h2t, *xs)
                self.flush_stair(keep=2 if PIPE else 0)
                if c1 is not None:
                    self._conv1_emit_upto(c1, 2 * ci + 2)
            if c1 is not None:
                h1t = c1["h1t"]
        self.flush_stair(keep=0)

        ysb = self.p_y.tile([self.NYC, 488], F32)
        nc.vector.tensor_copy(ysb[:], self.psum_y[0 : self.NYC, :])
        nc.sync.dma_start(
            out=_ap(self.y, 0, [(488, self.NYC), (1, 488)]), in_=ysb[:]
        )


_NC_CACHE = {}


def _get_nc():
    if "nc" not in _NC_CACHE:
        _NC_CACHE["nc"] = build_nc()
    return _NC_CACHE["nc"]


def _in_maps(inputs):
    x = np.ascontiguousarray(np.asarray(inputs["x"], dtype=np.float32))
    names = ["W1", "b1", "W2", "b2", "W3", "b3"]
    ws = {n: np.ascontiguousarray(np.asarray(inputs[n], np.float32)) for n in names}
    maps = []
    for i in range(8):
        m = {"x": x[i : i + 1]}
        m.update(ws)
        maps.append(m)
    return maps


def kernel(**inputs):
    nc = _get_nc()
    res = run_bass_kernel_spmd(nc, _in_maps(inputs), list(range(8)))
    return np.concatenate([res.results[i]["y"] for i in range(8)], axis=0)


def profile(**inputs):
    nc = _get_nc()
    res = run_bass_kernel_spmd(nc, _in_maps(inputs), list(range(8)), trace=True)
    return res.exec_time_ns


if __name__ == "__main__":
    rng = np.random.RandomState(0)
    ins = {
        "x": rng.randn(8, 1, H, W).astype(np.float32),
        "W1": rng.randn(CH, 1, 3, 3).astype(np.float32) * 0.1,
        "b1": np.zeros(CH, np.float32),
        "W2": rng.randn(CH, CH, 3, 3).astype(np.float32) * 0.05,
        "b2": np.zeros(CH, np.float32),
        "W3": rng.randn(C3, CH, 3, 3).astype(np.float32) * 0.05,
        "b3": np.zeros(C3, np.float32),
    }
    print(kernel(**ins).shape)
